# revision 19
# baseline (speedup 1.0000x reference)
"""Causal self-attention TRN2 kernel (8 NeuronCores), v2.

Problem: x[4,2048,1024] f32, w_qkv[3072,1024], w_proj[1024,1024]
  qkv = x @ w_qkv.T; per-head causal softmax(q k^T / sqrt(64)) v; out @ w_proj.T

Sharding: 8 cores = (head-group hg in {0,1}) x (batch b in {0..3}).
  Core computes its 8 heads for its batch; partial y (contracted over its
  512 channels of w_proj input dim) is summed pairwise on host.

v2 design (vs v1): single fused pass, all matmul operands bf16
(f32 psum accumulation), so the exp ACT stream (1 elem/lane/cycle
@1.2GHz - the co-bottleneck) hides under a dense PE stream:

  per t-chunk tcb (= attention i-block bi):
    qk-projection for the chunk (per head-pair, just-in-time),
    v-projection (reuses the same x chunk tile),
    attention j-loop (descending j):
      scores pair MM (2 heads row-tiled, concurrent)
      [diagonal tiles: causal mask folded into the scores PSUM group as
       a -240 bias MM (exp -> 0) + N-trimmed to the valid columns]
      exp via ACT psum->sbuf bf16
      PV: per-head M=64 MMs col-tiled to array halves (concurrent) +
          denominator row MMs (M=1) at tile_position (0,0)/(0,32),
          all four accumulating into 1.5 psum banks
    normalize: 2x reciprocal_approx_fast on the den rows, one gpsimd
      partition_broadcast [128,1024], 2 DVE muls -> attnT (bf16)
    out-projection of block bi queued as PE filler into block bi+1.

  Next-chunk qk / v-proj / prev-block proj matmuls are pumped from a
  pending queue between attention slots to keep PE busy (HAM warm).
"""

import numpy as np
import ml_dtypes

import concourse.bacc as bacc
import concourse.mybir as mybir
import concourse.tile as tile
from concourse.bass_utils import run_bass_kernel_spmd

F32 = mybir.dt.float32
BF16 = mybir.dt.bfloat16
EXP = mybir.ActivationFunctionType.Exp

B, T, C = 4, 2048, 1024
NH, HD = 16, 64
HPC = 8                      # heads per core
FH = HPC * HD                # 512: per-core q/k/v feature width
NCORES = 8
NKT = C // 128               # 8 contraction tiles
NTC = T // 512               # 4 t-chunks / i-blocks
LAG = 4                      # scores->PV software-pipeline depth (j-tiles)

_CACHE = {}


def build_nc():
    nc = bacc.Bacc()
    xT_d = nc.dram_tensor("xT", [C, T], BF16, kind="ExternalInput")
    wqkvT_d = nc.dram_tensor("wqkvT", [C, 3 * FH], BF16, kind="ExternalInput")
    wprojT_d = nc.dram_tensor("wprojT", [FH, C], BF16, kind="ExternalInput")
    cst_d = nc.dram_tensor("cst", [128, 384], BF16, kind="ExternalInput")
    y_d = nc.dram_tensor("y", [T, C], F32, kind="ExternalOutput")
    import os
    DBG = bool(os.environ.get("BASSDBG"))
    if DBG:
        dq_d = nc.dram_tensor("dbg_qk", [8 * 128, T], BF16, kind="ExternalOutput")
        dv_d = nc.dram_tensor("dbg_v", [16 * 128, HPC * 65], BF16, kind="ExternalOutput")
        dd_d = nc.dram_tensor("dbg_den", [16, 1024], F32, kind="ExternalOutput")
        da_d = nc.dram_tensor("dbg_at", [4 * 128, T], BF16, kind="ExternalOutput")

    with tile.TileContext(nc) as tc:
        with (
            tc.tile_pool(name="qkt", bufs=1) as qkt_pool,
            tc.tile_pool(name="vp", bufs=1) as v_pool,
            tc.tile_pool(name="at", bufs=1) as at_pool,
            tc.tile_pool(name="wq", bufs=1) as wq_pool,
            tc.tile_pool(name="wvp", bufs=1) as wv_pool,
            tc.tile_pool(name="wpj", bufs=1) as wp_pool,
            tc.tile_pool(name="cstp", bufs=1) as cst_pool,
            tc.tile_pool(name="xcp", bufs=1) as x_pool,
            tc.tile_pool(name="ptp", bufs=1) as pt_pool,
            tc.tile_pool(name="nrm", bufs=1) as nrm_pool,
            tc.tile_pool(name="otp", bufs=1) as ot_pool,
            tc.tile_pool(name="psS", bufs=1, space="PSUM") as psS,
            tc.tile_pool(name="psPV", bufs=1, space="PSUM") as psPV,
            tc.tile_pool(name="psD", bufs=1, space="PSUM") as psD,
            tc.tile_pool(name="psM", bufs=1, space="PSUM") as psM,
        ):
            qkT = [qkt_pool.tile([128, T], BF16, tag=f"qkt{i}", name=f"qkt{i}")
                   for i in range(8)]
            v_sb = [v_pool.tile([128, HPC * 65], BF16, tag=f"v{i}",
                            name=f"v{i}") for i in range(4 * NTC)]
            attnT = [at_pool.tile([128, T], BF16, tag=f"at{g}", name=f"at{g}")
                     for g in range(4)]
            wqk = wq_pool.tile([128, NKT * 1024], BF16, tag="wqk", name="wqk")
            wv = wv_pool.tile([128, NKT * FH], BF16, tag="wv", name="wv")
            wpj = wp_pool.tile([128, 4 * C], BF16, tag="wpj", name="wpj")
            cst = cst_pool.tile([128, 384], BF16, tag="cst", name="cst")
            ident = cst[:, 0:128]

            # prewarm the ACT exp table (first ACTIVATE otherwise pays the
            # ~2.7us PSEUDO_LOAD_ACT_FUNC_SET inside the attention loop)
            warm = nrm_pool.tile([1, 8], F32, tag="warm", name="warm")
            nc.vector.memset(warm[:], 0.0)
            nc.scalar.activation(warm[0:1, :], warm[0:1, :], EXP, scale=1.0)

            # ---- initial DMAs: sync ring = critical path, scalar = bulk ----
            wqk_v = wqk[:].rearrange("p (k f) -> p k f", k=NKT)
            xfull = x_pool.tile([128, NKT * T], BF16, tag="xf", name="xf")
            xf_v = xfull[:].rearrange("p (k t) -> p k t", k=NKT)
            xc0_v = xf_v[:, :, 0:512]
            nc.sync.dma_start(
                out=xc0_v[:, 0:4, :],
                in_=xT_d[0:512, 0:512].rearrange("(k p) t -> p k t", p=128))
            nc.sync.dma_start(
                out=wqk_v[:, 0:4, :],
                in_=wqkvT_d[0:512, 0:1024].rearrange("(k p) f -> p k f", p=128))
            nc.scalar.dma_start(out=cst[:], in_=cst_d[:, :])
            nc.scalar.dma_start(
                out=xc0_v[:, 4:NKT, :],
                in_=xT_d[512:C, 0:512].rearrange("(k p) t -> p k t", p=128))
            nc.scalar.dma_start(
                out=wqk_v[:, 4:NKT, :],
                in_=wqkvT_d[512:C, 0:1024].rearrange("(k p) f -> p k f", p=128))
            nc.sync.dma_start(
                out=wv[:].rearrange("p (k f) -> p k f", k=NKT),
                in_=wqkvT_d[0:C, 2 * FH:3 * FH].rearrange(
                    "(k p) f -> p k f", p=128))
            nc.scalar.dma_start(
                out=xf_v[:, :, 512:T],
                in_=xT_d[0:C, 512:T].rearrange("(k p) t -> p k t", p=128))
            nc.scalar.dma_start(
                out=wpj[:].rearrange("p (g f) -> p g f", g=4),
                in_=wprojT_d[0:FH, :].rearrange("(g p) f -> p g f", p=128))
            # PE warmup during the startup DMA wait: keeps HAM from
            # starting the real stream cold (cst arrives in ~3us)
            wps = psM.tile([128, 384], F32, tag="mmA", bufs=2, name="wps")
            for w in range(14):
                nc.tensor.matmul(wps[:], cst[:, 0:128], cst[:, :],
                                 start=(w == 0), stop=(w == 13))

            # ---------------- emitters ----------------
            def emit_qk_fi(tcb, fi):
                ps = psM.tile([128, 512], F32, tag="mmA", bufs=2,
                              name=f"psqk{tcb}_{fi}")
                for k in range(NKT):
                    nc.tensor.matmul(
                        ps[:],
                        wqk[:, k * 1024 + fi * 128:k * 1024 + (fi + 1) * 128],
                        xfull[:, k * T + tcb * 512:k * T + (tcb + 1) * 512],
                        start=(k == 0), stop=(k == NKT - 1))
                nc.vector.tensor_copy(
                    out=qkT[fi][:, tcb * 512:(tcb + 1) * 512], in_=ps[:])

            def emit_v(tcb, ti):
                ps = psM.tile([128, 512], F32, tag="mmA", bufs=2,
                              name=f"psv{ti}")
                for k in range(NKT):
                    nc.tensor.matmul(
                        ps[:],
                        xfull[:, k * T + ti * 128:k * T + (ti + 1) * 128],
                        wv[:, k * FH:(k + 1) * FH],
                        start=(k == 0), stop=(k == NKT - 1))
                vt = v_sb[ti]
                nc.vector.memset(vt[:], 1.0)
                nc.vector.tensor_copy(
                    out=vt[:].rearrange("p (h x) -> p h x", h=HPC)[:, :, 0:64],
                    in_=ps[:].rearrange("p (h x) -> p h x", h=HPC))

            def emit_proj(ti, fc):
                ps = psM.tile([128, 512], F32, tag="mmA", bufs=2,
                              name=f"po{ti}_{fc}")
                for g in range(4):
                    nc.tensor.matmul(
                        ps[:],
                        attnT[g][:, ti * 128:(ti + 1) * 128],
                        wpj[:, g * C + fc * 512:g * C + (fc + 1) * 512],
                        start=(g == 0), stop=(g == 3))
                ot = ot_pool.tile([128, 512], F32, tag="ot", bufs=2,
                                  name=f"ot{ti}_{fc}")
                nc.vector.tensor_copy(out=ot[:], in_=ps[:])
                nc.scalar.dma_start(
                    out=y_d[ti * 128:(ti + 1) * 128,
                            fc * 512:(fc + 1) * 512],
                    in_=ot[:])

            # pending PE-filler queue: (tag, closure)
            pending = []

            def pump(n=1):
                for _ in range(n):
                    if pending:
                        pending.pop(0)[1]()

            def flush(tag=None):
                keep = []
                for tg, fn in pending:
                    if tag is None or tg == tag:
                        fn()
                    else:
                        keep.append((tg, fn))
                pending[:] = keep

            def attention_hp(bi, hp):
                njt = 4 * bi + 4
                qt, kt = qkT[hp], qkT[4 + hp]
                pv = psPV.tile([65, 1024], F32, tag="pv", bufs=1,
                               name=f"pv{bi}_{hp}")
                pts = {}

                def emit_scores(jj):
                    sps = psS.tile([128, 1024], F32, tag="sps", bufs=2,
                                   name=f"sps{bi}_{hp}_{jj}")
                    r0 = jj - 4 * bi
                    lo = 128 * r0 if r0 >= 0 else 0
                    if r0 >= 0:
                        # causal bias: copy the -240 strictly-lower-tri
                        # pattern through PE into both par halves, then
                        # accumulate the diagonal-square scores on top;
                        # the region right of the square starts fresh.
                        for par in range(2):
                            nc.tensor.matmul(
                                sps[:, par * 512 + lo:par * 512 + lo + 128],
                                ident,
                                cst[:, 128 + 128 * par:256 + 128 * par],
                                start=True, stop=False)
                        for par in range(2):
                            off = par * 64
                            nc.tensor.matmul(
                                sps[:, par * 512 + lo:par * 512 + lo + 128],
                                kt[off:off + 64, jj * 128:(jj + 1) * 128],
                                qt[off:off + 64,
                                   bi * 512 + lo:bi * 512 + lo + 128],
                                start=False, stop=True)
                        if lo + 128 < 512:
                            for par in range(2):
                                off = par * 64
                                nc.tensor.matmul(
                                    sps[:, par * 512 + lo + 128:
                                        (par + 1) * 512],
                                    kt[off:off + 64, jj * 128:(jj + 1) * 128],
                                    qt[off:off + 64,
                                       bi * 512 + lo + 128:(bi + 1) * 512],
                                    start=True, stop=True)
                    else:
                        for par in range(2):
                            off = par * 64
                            nc.tensor.matmul(
                                sps[:, par * 512:(par + 1) * 512],
                                kt[off:off + 64, jj * 128:(jj + 1) * 128],
                                qt[off:off + 64, bi * 512:(bi + 1) * 512],
                                start=True, stop=True)
                    pt = pt_pool.tile([128, 1024], BF16, tag="pt",
                                      bufs=LAG + 2, name=f"pt{bi}_{hp}_{jj}")
                    if r0 >= 0 and lo > 0:
                        spsv = sps[:].rearrange("p (b i) -> p b i", b=2)
                        ptv = pt[:].rearrange("p (b i) -> p b i", b=2)
                        nc.scalar.activation(ptv[:, :, lo:512],
                                             spsv[:, :, lo:512], EXP,
                                             scale=0.125)
                    else:
                        nc.scalar.activation(pt[:], sps[:], EXP, scale=0.125)
                    pts[jj] = pt

                def emit_pv(jj, first, last):
                    r0 = jj - 4 * bi
                    lo = 128 * r0 if r0 >= 0 else 0
                    pt = pts.pop(jj)
                    for par in range(2):
                        h = 2 * hp + par
                        nc.tensor.matmul(
                            pv[:, par * 512 + lo:(par + 1) * 512],
                            v_sb[jj][:, h * 65:h * 65 + 65],
                            pt[:, par * 512 + lo:(par + 1) * 512],
                            start=first, stop=last)

                # ascending j: the first (start=True) MMs of the pv/den
                # accumulation chains cover the full column range; the
                # N-trimmed diagonal tiles come last
                for t in range(njt + LAG):
                    if t < njt:
                        emit_scores(t)
                    if t >= LAG:
                        jj = t - LAG
                        emit_pv(jj, first=(jj == 0), last=(jj == njt - 1))
                    pump(1)

                # normalize: pv rows scaled by 1/den along queries
                pvc = nrm_pool.tile([65, 1024], F32, tag="pvc", bufs=2,
                                    name=f"pvc{bi}_{hp}")
                nc.vector.tensor_copy(out=pvc[:, :], in_=pv[:, :])
                dsb = nrm_pool.tile([1, 1024], F32, tag="dsb", bufs=2,
                                    name=f"dsb{bi}_{hp}")
                nc.vector.tensor_copy(out=dsb[0:1, :], in_=pvc[64:65, :])
                if DBG:
                    nc.sync.dma_start(out=dd_d[4 * bi + hp:4 * bi + hp + 1, :],
                                      in_=dsb[0:1, :])
                rec = nrm_pool.tile([1, 1024], F32, tag="rec", bufs=2,
                                    name=f"rec{bi}_{hp}")
                nc.vector.reciprocal_approx_fast(out=rec[0:1, :],
                                                 in_=dsb[0:1, :])
                bc = nrm_pool.tile([128, 1024], F32, tag="bc", bufs=2,
                                   name=f"bc{bi}_{hp}")
                nc.gpsimd.partition_broadcast(bc[:, :], rec[0:1, :])
                nc.vector.tensor_mul(
                    out=attnT[hp][0:64, bi * 512:(bi + 1) * 512],
                    in0=pvc[0:64, 0:512], in1=bc[0:64, 0:512])
                nc.vector.tensor_mul(
                    out=attnT[hp][64:128, bi * 512:(bi + 1) * 512],
                    in0=pvc[0:64, 512:1024], in1=bc[0:64, 512:1024])

            # ---------------- main fused loop ----------------
            projq = []
            for tcb in range(NTC):
                bi = tcb
                flush("qknext")
                if tcb == 0:
                    emit_qk_fi(tcb, 0)
                    emit_qk_fi(tcb, 4)
                for hp in range(4):
                    if hp == 3 and tcb < NTC - 1:
                        pending.append(
                            ("qknext",
                             lambda tcb=tcb: emit_qk_fi(tcb + 1, 0)))
                        pending.append(
                            ("qknext",
                             lambda tcb=tcb: emit_qk_fi(tcb + 1, 4)))
                    if hp == 0:
                        # ascending: emit_v(ti) must precede emit_pv(jj=ti),
                        # which lands at slot (ti - 4*tcb) + LAG
                        for ti in range(4 * tcb, 4 * tcb + 4):
                            pending.append(
                                ("v", lambda tcb=tcb, ti=ti: emit_v(tcb, ti)))
                    if hp < 3:
                        pending.append(
                            (f"qk{hp + 1}",
                             lambda tcb=tcb, fi=hp + 1: emit_qk_fi(tcb, fi)))
                        pending.append(
                            (f"qk{hp + 1}",
                             lambda tcb=tcb, fi=5 + hp: emit_qk_fi(tcb, fi)))
                    nmove = 1 if tcb == NTC - 1 else 2
                    for _ in range(nmove):
                        if projq:
                            pending.append(projq.pop(0))
                    flush(f"qk{hp}")   # normally a no-op
                    attention_hp(bi, hp)
                for ti in range(4 * bi, 4 * bi + 4):
                    for fc in range(2):
                        projq.append(
                            ("proj", lambda ti=ti, fc=fc: emit_proj(ti, fc)))
            flush()
            for _, fn in projq:
                fn()
            if DBG:
                for i in range(8):
                    nc.sync.dma_start(out=dq_d[i * 128:(i + 1) * 128, :], in_=qkT[i][:])
                for i in range(16):
                    nc.sync.dma_start(out=dv_d[i * 128:(i + 1) * 128, :],
                                      in_=v_sb[i][:])
                for g in range(4):
                    nc.sync.dma_start(out=da_d[g * 128:(g + 1) * 128, :], in_=attnT[g][:])
    nc.compile()
    return nc


def _get_nc():
    if "nc" not in _CACHE:
        _CACHE["nc"] = build_nc()
    return _CACHE["nc"]


def kernel(x, w_qkv, w_proj, _trace=False):
    x = np.asarray(x, dtype=np.float32)
    w_qkv = np.asarray(w_qkv, dtype=np.float32)
    w_proj = np.asarray(w_proj, dtype=np.float32)
    BF = ml_dtypes.bfloat16

    nc = _get_nc()

    r = np.arange(128)
    patt = np.where(r[None, :] < r[:, None], -240.0, 0.0).astype(np.float32)
    cstnp = np.concatenate([np.eye(128, dtype=np.float32), patt, patt],
                           axis=1).astype(BF)

    in_maps = []
    for c in range(NCORES):
        hg, b = c // 4, c % 4
        xT = np.ascontiguousarray(x[b].T).astype(BF)          # [1024, 2048]
        rows = []
        for sec in range(3):                                   # q, k, v
            rows.append(w_qkv[sec * C + hg * FH: sec * C + (hg + 1) * FH])
        wqkvT = np.ascontiguousarray(np.concatenate(rows, 0).T).astype(BF)
        wprojT = np.ascontiguousarray(
            w_proj[:, hg * FH:(hg + 1) * FH].T).astype(BF)
        in_maps.append({"xT": xT, "wqkvT": wqkvT, "wprojT": wprojT,
                        "cst": cstnp})

    res = run_bass_kernel_spmd(nc, in_maps, list(range(NCORES)), trace=_trace)
    if _trace:
        _CACHE["exec_time_ns"] = res.exec_time_ns
        _CACHE["res"] = res

    y = np.empty((B, T, C), dtype=np.float32)
    for b in range(B):
        y[b] = res.results[b]["y"] + res.results[4 + b]["y"]
    return y


# revision 26
# speedup vs baseline: 1.0331x; 1.0331x over previous
"""Causal self-attention TRN2 kernel (8 NeuronCores), v2.

Problem: x[4,2048,1024] f32, w_qkv[3072,1024], w_proj[1024,1024]
  qkv = x @ w_qkv.T; per-head causal softmax(q k^T / sqrt(64)) v; out @ w_proj.T

Sharding: 8 cores = (head-group hg in {0,1}) x (batch b in {0..3}).
  Core computes its 8 heads for its batch; partial y (contracted over its
  512 channels of w_proj input dim) is summed pairwise on host.

v2 design (vs v1): single fused pass, all matmul operands bf16
(f32 psum accumulation), so the exp ACT stream (1 elem/lane/cycle
@1.2GHz - the co-bottleneck) hides under a dense PE stream:

  per t-chunk tcb (= attention i-block bi):
    qk-projection for the chunk (per head-pair, just-in-time),
    v-projection (reuses the persistent x tile),
    attention j-loop (ascending j, LAG-deep scores->PV pipeline):
      scores pair MM (2 heads row-tiled via base partition, concurrent)
      [diagonal tiles: causal mask folded into the scores PSUM group as
       a -240 bias MM (exp -> 0), N-trimmed to the valid columns; the
       region right of the diagonal square is a fresh start=True MM]
      exp via ACT psum->sbuf bf16 (N-trimmed on diagonal tiles)
      PV with [V|1] lhsT (M=65): denominator row rides free in row 64
    normalize: pv psum staged out via copy (releases the bank), then
      reciprocal_approx_fast -> gpsimd partition_broadcast [128,1024]
      -> 2 DVE muls -> attnT (bf16)
    out-projection of block bi queued as PE filler into block bi+1.

  Next-chunk qk / v-proj / prev-block proj matmuls are pumped from a
  pending queue between attention slots to keep PE busy (HAM warm).
  All input tensors are host-pre-tiled to SBUF layout so every DMA is
  a contiguous 2D copy (descriptor-efficient startup).

  Pitfalls baked in (cost a debugging round each): PSUM has_written is
  per-element on HW - every accumulation chain's first MM must cover
  the full element range it will ever touch with start=True; custom
  DVE ops (reciprocal_approx_fast) must not partition-shift their
  input (tensor_copy first); emit order defines Tile deps - a consumer
  emitted before its producer silently reads stale SBUF.
"""

import numpy as np
import ml_dtypes

import concourse.bacc as bacc
import concourse.mybir as mybir
import concourse.tile as tile
from concourse.bass_utils import run_bass_kernel_spmd

F32 = mybir.dt.float32
BF16 = mybir.dt.bfloat16
EXP = mybir.ActivationFunctionType.Exp

B, T, C = 4, 2048, 1024
NH, HD = 16, 64
HPC = 8                      # heads per core
FH = HPC * HD                # 512: per-core q/k/v feature width
NCORES = 8
NKT = C // 128               # 8 contraction tiles
NTC = T // 512               # 4 t-chunks / i-blocks
LAG = 3                      # scores->PV software-pipeline depth (j-tiles)

_CACHE = {}


def build_nc():
    nc = bacc.Bacc()
    # host-tiled: xT_d[p, tc, k, t512]; wqk_d[p, k, f1024];
    # wv_d[p, k, f512]; wpj_d[p, g, f1024] - all DMAs are contiguous copies
    xT_d = nc.dram_tensor("xT", [128, NTC * NKT * 512], BF16,
                          kind="ExternalInput")
    wqk_d = nc.dram_tensor("wqk", [128, NKT * 1024], BF16,
                           kind="ExternalInput")
    wv_d = nc.dram_tensor("wv", [128, NKT * FH], BF16, kind="ExternalInput")
    wpj_d = nc.dram_tensor("wpj", [128, 4 * C], BF16, kind="ExternalInput")
    cst_d = nc.dram_tensor("cst", [128, 384], BF16, kind="ExternalInput")
    y_d = nc.dram_tensor("y", [T, C], F32, kind="ExternalOutput")
    import os
    DBG = bool(os.environ.get("BASSDBG"))
    if DBG:
        dq_d = nc.dram_tensor("dbg_qk", [8 * 128, T], BF16, kind="ExternalOutput")
        dv_d = nc.dram_tensor("dbg_v", [16 * 128, HPC * 65], BF16, kind="ExternalOutput")
        dd_d = nc.dram_tensor("dbg_den", [16, 1024], F32, kind="ExternalOutput")
        da_d = nc.dram_tensor("dbg_at", [4 * 128, T], BF16, kind="ExternalOutput")

    with tile.TileContext(nc) as tc:
        with (
            tc.tile_pool(name="qkt", bufs=1) as qkt_pool,
            tc.tile_pool(name="vp", bufs=1) as v_pool,
            tc.tile_pool(name="at", bufs=1) as at_pool,
            tc.tile_pool(name="wq", bufs=1) as wq_pool,
            tc.tile_pool(name="wvp", bufs=1) as wv_pool,
            tc.tile_pool(name="wpj", bufs=1) as wp_pool,
            tc.tile_pool(name="cstp", bufs=1) as cst_pool,
            tc.tile_pool(name="xcp", bufs=1) as x_pool,
            tc.tile_pool(name="ptp", bufs=1) as pt_pool,
            tc.tile_pool(name="nrm", bufs=1) as nrm_pool,
            tc.tile_pool(name="otp", bufs=1) as ot_pool,
            tc.tile_pool(name="psS", bufs=1, space="PSUM") as psS,
            tc.tile_pool(name="psPV", bufs=1, space="PSUM") as psPV,
            tc.tile_pool(name="psD", bufs=1, space="PSUM") as psD,
            tc.tile_pool(name="psM", bufs=1, space="PSUM") as psM,
        ):
            qkT = [qkt_pool.tile([128, T], BF16, tag=f"qkt{i}", name=f"qkt{i}")
                   for i in range(8)]
            v_sb = [v_pool.tile([128, HPC * 65], BF16, tag=f"v{i}",
                            name=f"v{i}") for i in range(4 * NTC)]
            attnT = [at_pool.tile([128, T], BF16, tag=f"at{g}", name=f"at{g}")
                     for g in range(4)]
            wqk = wq_pool.tile([128, NKT * 1024], BF16, tag="wqk", name="wqk")
            wv = wv_pool.tile([128, NKT * FH], BF16, tag="wv", name="wv")
            wpj = wp_pool.tile([128, 4 * C], BF16, tag="wpj", name="wpj")
            cst = cst_pool.tile([128, 384], BF16, tag="cst", name="cst")
            ident = cst[:, 0:128]

            # prewarm the ACT exp table (first ACTIVATE otherwise pays the
            # ~2.7us PSEUDO_LOAD_ACT_FUNC_SET inside the attention loop)
            warm = nrm_pool.tile([1, 8], F32, tag="warm", name="warm")
            nc.vector.memset(warm[:], 0.0)
            nc.scalar.activation(warm[0:1, :], warm[0:1, :], EXP, scale=1.0)

            # ---- initial DMAs: sync ring = critical path, scalar = bulk ----
            xfull = x_pool.tile([128, NKT * T], BF16, tag="xf", name="xf")
            xf_v = xfull[:].rearrange("p (k t) -> p k t", k=NKT)
            xc0_v = xf_v[:, :, 0:512]
            nc.sync.dma_start(out=xc0_v[:, 0:4, :],
                              in_=xT_d[:, 0:2048])
            nc.sync.dma_start(out=wqk[:, 0:2048], in_=wqk_d[:, 0:2048])
            nc.scalar.dma_start(out=cst[:], in_=cst_d[:, :])
            nc.scalar.dma_start(out=xc0_v[:, 4:NKT, :],
                                in_=xT_d[:, 2048:4096])
            nc.scalar.dma_start(out=wqk[:, 2048:NKT * 1024],
                                in_=wqk_d[:, 2048:NKT * 1024])
            nc.sync.dma_start(out=wv[:], in_=wv_d[:, :])
            for tc_ in range(1, NTC):
                nc.scalar.dma_start(
                    out=xf_v[:, :, tc_ * 512:(tc_ + 1) * 512],
                    in_=xT_d[:, tc_ * 4096:(tc_ + 1) * 4096].rearrange(
                        "p (k t) -> p k t", t=512))
            nc.scalar.dma_start(out=wpj[:], in_=wpj_d[:, :])
            # PE warmup during the startup DMA wait: keeps HAM from
            # starting the real stream cold (cst arrives in ~3us)
            wps = psM.tile([128, 384], F32, tag="mmA", bufs=2, name="wps")
            for w in range(30):
                nc.tensor.matmul(wps[:], cst[:, 0:128], cst[:, :],
                                 start=(w == 0), stop=(w == 29))

            # ---------------- emitters ----------------
            def emit_qk_fi(tcb, fi):
                # wqk is [p, pos, k, 128] with pos ordered [q0,k0,q1,k1,...]
                # so each head-pair's q+k weights are a contiguous 512KB
                pos = 2 * (fi % 4) + (fi // 4)
                ps = psM.tile([128, 512], F32, tag="mmA", bufs=2,
                              name=f"psqk{tcb}_{fi}")
                for k in range(NKT):
                    nc.tensor.matmul(
                        ps[:],
                        wqk[:, pos * 1024 + k * 128:pos * 1024 + (k + 1) * 128],
                        xfull[:, k * T + tcb * 512:k * T + (tcb + 1) * 512],
                        start=(k == 0), stop=(k == NKT - 1))
                nc.vector.tensor_copy(
                    out=qkT[fi][:, tcb * 512:(tcb + 1) * 512], in_=ps[:])

            def emit_v(tcb, ti):
                ps = psM.tile([128, 512], F32, tag="mmA", bufs=2,
                              name=f"psv{ti}")
                for k in range(NKT):
                    nc.tensor.matmul(
                        ps[:],
                        xfull[:, k * T + ti * 128:k * T + (ti + 1) * 128],
                        wv[:, k * FH:(k + 1) * FH],
                        start=(k == 0), stop=(k == NKT - 1))
                vt = v_sb[ti]
                nc.vector.memset(vt[:], 1.0)
                nc.vector.tensor_copy(
                    out=vt[:].rearrange("p (h x) -> p h x", h=HPC)[:, :, 0:64],
                    in_=ps[:].rearrange("p (h x) -> p h x", h=HPC))

            def emit_proj(ti, fc, alt=False):
                if alt:
                    # tail-only: attention is done, reuse the idle score
                    # psum banks so the drains of consecutive groups overlap
                    ps = psS.tile([128, 1024], F32, tag="sps", bufs=2,
                                  name=f"po{ti}_{fc}")[:, 0:512]
                else:
                    ps = psM.tile([128, 512], F32, tag="mmA", bufs=2,
                                  name=f"po{ti}_{fc}")
                for g in range(4):
                    nc.tensor.matmul(
                        ps[:],
                        attnT[g][:, ti * 128:(ti + 1) * 128],
                        wpj[:, g * C + fc * 512:g * C + (fc + 1) * 512],
                        start=(g == 0), stop=(g == 3))
                ot = ot_pool.tile([128, 512], F32, tag="ot", bufs=2,
                                  name=f"ot{ti}_{fc}")
                nc.vector.tensor_copy(out=ot[:], in_=ps[:])
                nc.scalar.dma_start(
                    out=y_d[ti * 128:(ti + 1) * 128,
                            fc * 512:(fc + 1) * 512],
                    in_=ot[:])

            # pending PE-filler queue: (tag, closure)
            pending = []

            def pump(n=1):
                for _ in range(n):
                    if pending:
                        pending.pop(0)[1]()

            def flush(tag=None):
                keep = []
                for tg, fn in pending:
                    if tag is None or tg == tag:
                        fn()
                    else:
                        keep.append((tg, fn))
                pending[:] = keep

            def attention_hp(bi, hp):
                njt = 4 * bi + 4
                qt, kt = qkT[hp], qkT[4 + hp]
                pv = psPV.tile([65, 1024], F32, tag="pv", bufs=1,
                               name=f"pv{bi}_{hp}")
                pts = {}

                def emit_scores(jj):
                    sps = psS.tile([128, 1024], F32, tag="sps", bufs=2,
                                   name=f"sps{bi}_{hp}_{jj}")
                    r0 = jj - 4 * bi
                    lo = 128 * r0 if r0 >= 0 else 0
                    if r0 >= 0:
                        # causal bias: copy the -240 strictly-lower-tri
                        # pattern through PE into both par halves, then
                        # accumulate the diagonal-square scores on top;
                        # the region right of the square starts fresh.
                        for par in range(2):
                            nc.tensor.matmul(
                                sps[:, par * 512 + lo:par * 512 + lo + 128],
                                ident,
                                cst[:, 128 + 128 * par:256 + 128 * par],
                                start=True, stop=False)
                        for par in range(2):
                            off = par * 64
                            nc.tensor.matmul(
                                sps[:, par * 512 + lo:par * 512 + lo + 128],
                                kt[off:off + 64, jj * 128:(jj + 1) * 128],
                                qt[off:off + 64,
                                   bi * 512 + lo:bi * 512 + lo + 128],
                                start=False, stop=True)
                        if lo + 128 < 512:
                            for par in range(2):
                                off = par * 64
                                nc.tensor.matmul(
                                    sps[:, par * 512 + lo + 128:
                                        (par + 1) * 512],
                                    kt[off:off + 64, jj * 128:(jj + 1) * 128],
                                    qt[off:off + 64,
                                       bi * 512 + lo + 128:(bi + 1) * 512],
                                    start=True, stop=True)
                    else:
                        for par in range(2):
                            off = par * 64
                            nc.tensor.matmul(
                                sps[:, par * 512:(par + 1) * 512],
                                kt[off:off + 64, jj * 128:(jj + 1) * 128],
                                qt[off:off + 64, bi * 512:(bi + 1) * 512],
                                start=True, stop=True)
                    pt = pt_pool.tile([128, 1024], BF16, tag="pt",
                                      bufs=LAG + 2, name=f"pt{bi}_{hp}_{jj}")
                    if r0 >= 0 and lo > 0:
                        spsv = sps[:].rearrange("p (b i) -> p b i", b=2)
                        ptv = pt[:].rearrange("p (b i) -> p b i", b=2)
                        nc.scalar.activation(ptv[:, :, lo:512],
                                             spsv[:, :, lo:512], EXP,
                                             scale=0.125)
                    else:
                        nc.scalar.activation(pt[:], sps[:], EXP, scale=0.125)
                    pts[jj] = pt

                def emit_pv(jj, first, last):
                    r0 = jj - 4 * bi
                    lo = 128 * r0 if r0 >= 0 else 0
                    pt = pts.pop(jj)
                    for par in range(2):
                        h = 2 * hp + par
                        nc.tensor.matmul(
                            pv[:, par * 512 + lo:(par + 1) * 512],
                            v_sb[jj][:, h * 65:h * 65 + 65],
                            pt[:, par * 512 + lo:(par + 1) * 512],
                            start=first, stop=last)

                # ascending j: the first (start=True) MMs of the pv/den
                # accumulation chains cover the full column range; the
                # N-trimmed diagonal tiles come last
                for t in range(njt + LAG):
                    if t < njt:
                        emit_scores(t)
                    if t >= LAG:
                        jj = t - LAG
                        emit_pv(jj, first=(jj == 0), last=(jj == njt - 1))
                    pump(1)

                # normalize: pv rows scaled by 1/den along queries
                pvc = nrm_pool.tile([65, 1024], F32, tag="pvc", bufs=2,
                                    name=f"pvc{bi}_{hp}")
                nc.vector.tensor_copy(out=pvc[:, :], in_=pv[:, :])
                dsb = nrm_pool.tile([1, 1024], F32, tag="dsb", bufs=2,
                                    name=f"dsb{bi}_{hp}")
                nc.vector.tensor_copy(out=dsb[0:1, :], in_=pvc[64:65, :])
                if DBG:
                    nc.sync.dma_start(out=dd_d[4 * bi + hp:4 * bi + hp + 1, :],
                                      in_=dsb[0:1, :])
                rec = nrm_pool.tile([1, 1024], F32, tag="rec", bufs=2,
                                    name=f"rec{bi}_{hp}")
                nc.vector.reciprocal_approx_fast(out=rec[0:1, :],
                                                 in_=dsb[0:1, :])
                bc = nrm_pool.tile([128, 1024], F32, tag="bc", bufs=2,
                                   name=f"bc{bi}_{hp}")
                nc.gpsimd.partition_broadcast(bc[:, :], rec[0:1, :])
                nc.vector.tensor_mul(
                    out=attnT[hp][0:64, bi * 512:(bi + 1) * 512],
                    in0=pvc[0:64, 0:512], in1=bc[0:64, 0:512])
                nc.vector.tensor_mul(
                    out=attnT[hp][64:128, bi * 512:(bi + 1) * 512],
                    in0=pvc[0:64, 512:1024], in1=bc[0:64, 512:1024])

            # ---------------- main fused loop ----------------
            projq = []
            for tcb in range(NTC):
                bi = tcb
                flush("qknext")
                if tcb == 0:
                    emit_qk_fi(tcb, 0)
                    emit_qk_fi(tcb, 4)
                for hp in range(4):
                    if hp == 3 and tcb < NTC - 1:
                        pending.append(
                            ("qknext",
                             lambda tcb=tcb: emit_qk_fi(tcb + 1, 0)))
                        pending.append(
                            ("qknext",
                             lambda tcb=tcb: emit_qk_fi(tcb + 1, 4)))
                    if hp == 0:
                        # ascending: emit_v(ti) must precede emit_pv(jj=ti),
                        # which lands at slot (ti - 4*tcb) + LAG
                        for ti in range(4 * tcb, 4 * tcb + 4):
                            pending.append(
                                ("v", lambda tcb=tcb, ti=ti: emit_v(tcb, ti)))
                    if hp < 3:
                        pending.append(
                            (f"qk{hp + 1}",
                             lambda tcb=tcb, fi=hp + 1: emit_qk_fi(tcb, fi)))
                        pending.append(
                            (f"qk{hp + 1}",
                             lambda tcb=tcb, fi=5 + hp: emit_qk_fi(tcb, fi)))
                    nmove = 1 if tcb == NTC - 1 else 2
                    for _ in range(nmove):
                        if projq:
                            pending.append(projq.pop(0))
                    flush(f"qk{hp}")   # normally a no-op
                    attention_hp(bi, hp)
                for ti in range(4 * bi, 4 * bi + 4):
                    for fc in range(2):
                        projq.append(
                            ("proj",
                             lambda ti=ti, fc=fc, **kw: emit_proj(ti, fc, **kw)))
            flush()
            for i, (_, fn) in enumerate(projq):
                fn(alt=(i % 2 == 1))
            if DBG:
                for i in range(8):
                    nc.sync.dma_start(out=dq_d[i * 128:(i + 1) * 128, :], in_=qkT[i][:])
                for i in range(16):
                    nc.sync.dma_start(out=dv_d[i * 128:(i + 1) * 128, :],
                                      in_=v_sb[i][:])
                for g in range(4):
                    nc.sync.dma_start(out=da_d[g * 128:(g + 1) * 128, :], in_=attnT[g][:])
    nc.compile()
    return nc


def _get_nc():
    if "nc" not in _CACHE:
        _CACHE["nc"] = build_nc()
    return _CACHE["nc"]


def kernel(x, w_qkv, w_proj, _trace=False):
    x = np.asarray(x, dtype=np.float32)
    w_qkv = np.asarray(w_qkv, dtype=np.float32)
    w_proj = np.asarray(w_proj, dtype=np.float32)
    BF = ml_dtypes.bfloat16

    nc = _get_nc()

    r = np.arange(128)
    patt = np.where(r[None, :] < r[:, None], -240.0, 0.0).astype(np.float32)
    cstnp = np.concatenate([np.eye(128, dtype=np.float32), patt, patt],
                           axis=1).astype(BF)

    in_maps = []
    for c in range(NCORES):
        hg, b = c // 4, c % 4
        xT = x[b].T.astype(BF)                                 # [1024, 2048]
        # [p, tc, k, t512]
        xt_t = np.ascontiguousarray(
            xT.reshape(8, 128, 4, 512).transpose(1, 2, 0, 3)).reshape(128, -1)
        rows = []
        for sec in range(3):                                   # q, k, v
            rows.append(w_qkv[sec * C + hg * FH: sec * C + (hg + 1) * FH])
        wqkvT = np.concatenate(rows, 0).T.astype(BF)           # [1024, 1536]
        idx = []
        for hp in range(4):                       # pos order [q0,k0,q1,k1..]
            idx += list(range(hp * 128, (hp + 1) * 128))
            idx += list(range(512 + hp * 128, 512 + (hp + 1) * 128))
        wqk_t = np.ascontiguousarray(
            wqkvT[:, 0:1024][:, idx].reshape(8, 128, 8, 128)
            .transpose(1, 2, 0, 3)).reshape(128, -1)
        wv_t = np.ascontiguousarray(
            wqkvT[:, 1024:1536].reshape(8, 128, 512).transpose(1, 0, 2)
        ).reshape(128, -1)
        wprojT = w_proj[:, hg * FH:(hg + 1) * FH].T.astype(BF)  # [512, 1024]
        wpj_t = np.ascontiguousarray(
            wprojT.reshape(4, 128, 1024).transpose(1, 0, 2)).reshape(128, -1)
        in_maps.append({"xT": xt_t, "wqk": wqk_t, "wv": wv_t, "wpj": wpj_t,
                        "cst": cstnp})

    res = run_bass_kernel_spmd(nc, in_maps, list(range(NCORES)), trace=_trace)
    if _trace:
        _CACHE["exec_time_ns"] = res.exec_time_ns
        _CACHE["res"] = res

    y = np.empty((B, T, C), dtype=np.float32)
    for b in range(B):
        y[b] = res.results[b]["y"] + res.results[4 + b]["y"]
    return y
